# revision 18
# baseline (speedup 1.0000x reference)
"""Damped electrostatics (charge+dipole+quadrupole, switched) over 3.2M edges
on 8 Trainium2 NeuronCores.

Strategy (data-parallel over edges):
  - Shard the [E]-indexed tensors across the 8 cores (400k edges each).
  - Per-edge u/v atom records are resolved during host-side sharding into
    planar per-edge streams (device indirect-DMA gathers cost ~1.4us per 128
    records on this HW -- far off the roofline; streaming planar operands
    is the only way to feed the DVE at rate).
  - All device math runs in fp16: every DVE tensor_tensor ALU op qualifies
    for the 2x_1p perf mode (2-byte packed operands -> 0.5 cycle/elem) and
    DMA bytes halve.  Tolerance is 2e-2 vs a measured fp32 error of ~3e-6,
    so fp16 (~1e-3 elementwise) has ample margin.  Ln/Exp intermediates
    (L = ln d) stay fp32 on the ACT engine: Exp(-5L) amplifies input error
    5x and fp16 quantization of L would cost ~1% there.
  - Edges are sorted by distance within each core (the slot->edge mapping is
    inverted on unshard).  With ascending d all d<2 edges land in tile 0:
    only that tile evaluates the quintic switch / damped-Coulomb blend
    (exact for d>=2 too, so tile-0 overflow slots are still correct).
    Tiles 1..4 use chi = 1/d via the ACT Ln/Exp tables.  Only the last tile
    needs the d <= CUTOFF mask (largest d sorts there).
  - Constant folding: sqrt(KEHALF) is folded into the per-atom charge/
    dipole/quadrupole tables on the host, so every per-edge u*v product
    carries KEHALF automatically.  The charge-term 2x (from qu2 = 2*qu,
    needed by the dipole term) is cancelled by folding ln(0.5) into the
    Exp biases of r1 and r5.  The quadrupole table is pre-reduced per atom:
    B = sym(Q) - (tr(Q)/3) I with off-diagonals doubled, so the per-edge
    quadrupole contraction is v^T B v (6 products).
  - Vector-engine work is issued as wide block ops over contiguous 3-plane
    groups ([128, 3W] per instruction) to amortize per-instruction
    overhead; tiles are uneven (one 400-wide switch tile, four 700-wide
    fast tiles) so the expensive switch path only covers the columns that
    need it.  GPSIMD is intentionally NOT used: it contends with DVE for
    SBUF ports (measured ~40% slowdown of concurrent DVE ops).
"""

import os
import sys

for _p in ("/opt/trn_rl_repo", "/root/.axon_site/_ro/trn_rl_repo"):
    if os.path.isdir(_p) and _p not in sys.path:
        sys.path.append(_p)

import numpy as np

import concourse.bass as bass
import concourse.mybir as mybir
import concourse.tile as tile
from concourse.bass_utils import run_bass_kernel_spmd

F32 = mybir.dt.float32
F16 = mybir.dt.float16
ALU = mybir.AluOpType
ACT = mybir.ActivationFunctionType

N_CORES = 8
N_ATOMS = 100000
N_EDGES = 3200000
E_CORE = N_EDGES // N_CORES          # 400000
P = 128
W_TOT = 3008                         # computed columns; 385024 slots hold the
                                     # 385k nearest edges (d<=12 edges are
                                     # ~383.4k/core, 12-sigma margin); the
                                     # d>12 tail is emitted as exact zeros on
                                     # the host and never reaches the device
TILE_W = [400, 870, 869, 869]        # tile 0 = switch region (d < 2)
N_PLANES = 18
LN_HALF = -0.6931471805599453

CUTOFF = 12.0
KEHALF = 7.199822675975274

_MAX_WAITS = 1  # this walrus build allows only 1 sync wait on some instruction types


def _split_sync_waits(nc):
    """Walrus here fails codegen ("Too many sync wait commands") for any
    instruction carrying more than _MAX_WAITS semaphore waits. Move excess
    waits onto same-engine NOPs inserted immediately before the instruction:
    the sequencer executes waits in program order, so this is equivalent."""
    import bass_rust

    counter = [0]
    for fn in nc.m.functions:
        for bb in fn.blocks:
            insts = list(bb.instructions)
            out = []
            changed = False
            for inst in insts:
                si = inst.sync_info
                waits = list(si.on_wait) if (si and si.on_wait) else []
                if len(waits) > _MAX_WAITS:
                    changed = True
                    head, rest = waits[:-_MAX_WAITS], waits[-_MAX_WAITS:]
                    for i in range(0, len(head), _MAX_WAITS):
                        counter[0] += 1
                        nop = bass_rust.InstNoOp(
                            name=f"I-waitsplit-{counter[0]}", ins=[], outs=[]
                        )
                        nop.engine = inst.engine
                        nop.sync_info = mybir.SyncInfo(
                            on_wait=head[i:i + _MAX_WAITS], on_update=[]
                        )
                        out.append(nop)
                    si.on_wait = rest
                out.append(inst)
            if changed:
                bb.instructions = out


def _register_const(nc, value, dtype=F32):
    t = nc.alloc_sbuf_tensor(f"const-{dtype.name}-{value}", [128, 1], dtype)
    nc.gpsimd.memset(t.ap(), value)
    nc.const_aps.aps[(dtype, value)] = t.ap()


def _build_module():
    nc = bass.Bass()
    _register_const(nc, LN_HALF)
    nc.all_engine_barrier()

    # host packs per tile: [P, sum_t(18*W_t)] fp16, planes contiguous per tile
    total_cols = N_PLANES * W_TOT
    x_in = nc.dram_tensor("x", [P, total_cols], F16, kind="ExternalInput")
    out = nc.dram_tensor("out", [P, W_TOT], F16, kind="ExternalOutput")

    with tile.TileContext(nc) as tc:
        with (
            tc.tile_pool(name="io", bufs=3) as io_pool,
            tc.tile_pool(name="scr", bufs=2) as scr_pool,
        ):
            x_off = 0
            o_off = 0
            for it, W in enumerate(TILE_W):
                slow = it == 0                    # only tile 0 holds d < 2
                masked = it == len(TILE_W) - 1    # only last tile holds d > CUTOFF
                W3 = 3 * W

                # geometry planes (d, v) land first so the ACT chi chain and
                # the v-only DVE work start while the atom block streams in
                xt = io_pool.tile([P, N_PLANES * W], F16, tag="xt")
                nc.sync.dma_start(
                    out=xt[:, 0:4 * W], in_=x_in[:, x_off:x_off + 4 * W]
                )
                nc.sync.dma_start(
                    out=xt[:, 4 * W:10 * W],
                    in_=x_in[:, x_off + 4 * W:x_off + 10 * W],
                )
                nc.sync.dma_start(
                    out=xt[:, 10 * W:N_PLANES * W],
                    in_=x_in[:, x_off + 10 * W:x_off + N_PLANES * W],
                )
                x_off += N_PLANES * W

                def pl(k, n=1):
                    return xt[:, k * W:(k + n) * W]

                d = pl(0)
                vblk = pl(1, 3)
                wblk = pl(4, 3)
                ublk = pl(7, 3)
                qv = pl(10)
                qu2 = pl(11)
                bblk = pl(12, 3)
                cblk = pl(15, 3)
                v0, v1, v2 = pl(1), pl(2), pl(3)

                def scr(tag, w=W, dt=F16):
                    return scr_pool.tile([P, w], dt, tag=tag, name=tag)

                # --- chi powers ---------------------------------------------
                # r1 = 0.5*chi ; rA = chi^2/d (term1) ; rB = chi^3 ;
                # r5 = 0.5*chi^3/d^2.  Fast path (chi = 1/d): rA == rB.
                # ACT ops are issued first; the slow tile's DVE pieces of the
                # chi chain are deferred until after the (independent) dot-
                # product block so the in-order DVE queue never stalls on ACT.
                vq6 = scr("vq6", 6 * W)   # vsq (3W) | vp pair products (3W)
                vsq = vq6[:, 0:W3]
                nc.scalar.activation(vsq, vblk, ACT.Square)
                r1 = scr("r1")
                r3 = scr("r3")
                r5 = scr("r5")
                if slow:
                    x = scr("x")          # clip(d/2, 0, 1)
                    nc.vector.tensor_scalar(x[:], d, 0.5, 1.0, ALU.mult, ALU.min)
                    h = scr("h")          # 15 - 6x
                    nc.vector.tensor_scalar(
                        h[:], x[:], -6.0, 15.0, ALU.mult, ALU.add
                    )
                    sq = scr("sq", dt=F32)
                    nc.scalar.activation(sq[:], d, ACT.Square)
                    L2 = scr("L2", dt=F32)
                    nc.scalar.activation(L2[:], sq[:], ACT.Ln, bias=1.0)
                    x2 = scr("x2")
                    nc.scalar.activation(x2[:], x[:], ACT.Square)
                    ri = scr("ri")        # 0.5/sqrt(d^2+1)
                    nc.scalar.activation(
                        ri[:], L2[:], ACT.Exp, bias=LN_HALF, scale=-0.5
                    )
                    L = scr("L", dt=F32)
                    nc.scalar.activation(L[:], d, ACT.Ln)
                    r = scr("r")          # 0.5/d
                    nc.scalar.activation(
                        r[:], L[:], ACT.Exp, bias=LN_HALF, scale=-1.0
                    )
                    rr2 = scr("rr2")      # 1/d^2 = (2*r)^2
                    nc.scalar.activation(rr2[:], r[:], ACT.Square, scale=2.0)
                else:
                    # d >= 2 -> sw == 0 -> chi = 1/d exactly (ACT Ln/Exp
                    # tables, ~1.3e-4 rel; tolerance is 2e-2)
                    L = scr("L", dt=F32)
                    nc.scalar.activation(L[:], d, ACT.Ln)
                    nc.scalar.activation(
                        r1[:], L[:], ACT.Exp, bias=LN_HALF, scale=-1.0
                    )
                    nc.scalar.activation(r3[:], L[:], ACT.Exp, scale=-3.0)
                    nc.scalar.activation(
                        r5[:], L[:], ACT.Exp, bias=LN_HALF, scale=-5.0
                    )
                # --- dot products as 3-plane block ops ----------------------
                # liveness-driven in-place: pm first (consumes u,w as pure
                # reads), then the v.w / v.u products overwrite w / u blocks
                pm = scr("pm", W3)        # mu_u .* mu_v
                nc.vector.tensor_tensor(pm[:], ublk, wblk, ALU.mult)
                nc.vector.tensor_tensor(wblk, vblk, wblk, ALU.mult)  # v.*mu_v
                nc.vector.tensor_tensor(ublk, vblk, ublk, ALU.mult)  # v.*mu_u
                sv = wblk[:, 0:W]         # KE * (v . mu_v)
                nc.vector.tensor_tensor(sv, wblk[:, 0:W], wblk[:, W:2 * W], ALU.add)
                nc.vector.tensor_tensor(sv, sv, wblk[:, 2 * W:W3], ALU.add)
                su = ublk[:, 0:W]         # v . mu_u (sqrt(KE) scale)
                nc.vector.tensor_tensor(su, ublk[:, 0:W], ublk[:, W:2 * W], ALU.add)
                nc.vector.tensor_tensor(su, su, ublk[:, 2 * W:W3], ALU.add)
                M = pm[:, 0:W]            # KE * (mu_u . mu_v)
                nc.vector.tensor_tensor(M, pm[:, 0:W], pm[:, W:2 * W], ALU.add)
                nc.vector.tensor_tensor(M, M, pm[:, 2 * W:W3], ALU.add)

                # --- quadrupole form wq = v^T B v ---------------------------
                vp = vq6[:, W3:6 * W]     # v0v1 | v0v2 | v1v2
                nc.vector.tensor_tensor(vp[:, 0:W], v0, v1, ALU.mult)
                nc.vector.tensor_tensor(vp[:, W:2 * W], v0, v2, ALU.mult)
                nc.vector.tensor_tensor(vp[:, 2 * W:W3], v1, v2, ALU.mult)
                bc6 = pl(12, 6)           # one 6W op: vsq*b | vp*c
                nc.vector.tensor_tensor(bc6, vq6[:], bc6, ALU.mult)
                nc.vector.tensor_tensor(cblk, bblk, cblk, ALU.add)
                wq = cblk[:, 0:W]
                nc.vector.tensor_tensor(wq, cblk[:, 0:W], cblk[:, W:2 * W], ALU.add)
                nc.vector.tensor_tensor(wq, wq, cblk[:, 2 * W:W3], ALU.add)

                if slow:
                    # deferred DVE pieces of the switch/blend chain (their
                    # ACT inputs completed during the product block)
                    x3 = scr("x3")
                    nc.vector.tensor_tensor(x3[:], x2[:], x[:], ALU.mult)
                    h2 = scr("h2")        # x*(15-6x)
                    nc.vector.tensor_tensor(h2[:], h[:], x[:], ALU.mult)
                    swm1 = scr("swm1")    # sw - 1 = (h2 - 10)*x^3
                    nc.vector.scalar_tensor_tensor(
                        swm1[:], h2[:], -10.0, x3[:], ALU.add, ALU.mult
                    )
                    rdif = scr("rdif")    # ri - r
                    nc.vector.tensor_tensor(rdif[:], ri[:], r[:], ALU.subtract)
                    # r1 = ri + (sw-1)*(ri-r) = 0.5*chi
                    nc.vector.tensor_tensor(r1[:], swm1[:], rdif[:], ALU.mult)
                    nc.vector.tensor_tensor(r1[:], r1[:], ri[:], ALU.add)
                    c2 = scr("c2")        # chi^2 = (2*r1)^2
                    nc.scalar.activation(c2[:], r1[:], ACT.Square, scale=2.0)
                    rA = scr("rA")        # chi^2/d = c2 * 2 * (0.5/d)
                    nc.vector.scalar_tensor_tensor(
                        rA[:], c2[:], 2.0, r[:], ALU.mult, ALU.mult
                    )
                    # rB = chi^3 = 2*c2*r1
                    nc.vector.scalar_tensor_tensor(
                        r3[:], c2[:], 2.0, r1[:], ALU.mult, ALU.mult
                    )
                    # r5 = 0.5*chi^3/d^2
                    nc.vector.scalar_tensor_tensor(
                        r5[:], r3[:], 0.5, rr2[:], ALU.mult, ALU.mult
                    )

                # --- assemble ----------------------------------------------
                # E = cq*r1 + (qu2*sv)*rA + M*rB + (qu2*wq - 6 sv su)*r5
                # (d > CUTOFF edges never reach the device; the host emits
                # exact zeros for them)
                e = qv                    # in-place: qv dead after first op
                nc.vector.tensor_tensor(e, qu2, qv, ALU.mult)
                nc.vector.tensor_tensor(e, e, r1[:], ALU.mult)
                t = scr("t")
                nc.vector.tensor_tensor(t[:], qu2, sv, ALU.mult)
                if slow:
                    nc.vector.tensor_tensor(t[:], t[:], rA[:], ALU.mult)
                    nc.vector.tensor_tensor(e, e, t[:], ALU.add)
                    nc.vector.tensor_tensor(t[:], M, r3[:], ALU.mult)
                    nc.vector.tensor_tensor(e, e, t[:], ALU.add)
                else:
                    nc.vector.tensor_tensor(t[:], t[:], M, ALU.add)
                    nc.vector.tensor_tensor(t[:], t[:], r3[:], ALU.mult)
                    nc.vector.tensor_tensor(e, e, t[:], ALU.add)
                nc.vector.tensor_tensor(wq, wq, qu2, ALU.mult)
                p = vp[:, 0:W]            # vp dead after the cblk product
                nc.vector.tensor_tensor(p, sv, su, ALU.mult)
                nc.vector.scalar_tensor_tensor(
                    p, p, -6.0, wq, ALU.mult, ALU.add
                )
                nc.vector.tensor_tensor(p, p, r5[:], ALU.mult)

                res = io_pool.tile([P, W], F16, tag="res")
                nc.vector.tensor_tensor(res[:], e, p, ALU.add)

                nc.sync.dma_start(out=out[:, o_off:o_off + W], in_=res[:])
                o_off += W

    return nc


def _prep_inputs(distances_uv, vectors_uv, atomic_charges, atomic_dipoles,
                 atomic_quadrupoles, idx_u, idx_v):
    d = np.ascontiguousarray(np.asarray(distances_uv, dtype=np.float32))
    vec = np.ascontiguousarray(np.asarray(vectors_uv, dtype=np.float32))
    q = np.asarray(atomic_charges, dtype=np.float32)
    mu = np.asarray(atomic_dipoles, dtype=np.float32)
    Q = np.asarray(atomic_quadrupoles, dtype=np.float32)
    iu = np.asarray(idx_u, dtype=np.int64)
    iv = np.asarray(idx_v, dtype=np.int64)

    rke = np.float32(np.sqrt(KEHALF))
    qs = rke * q                      # sqrt(KE) * q
    qs2 = 2.0 * qs                    # 2 sqrt(KE) * q
    mus = rke * mu                    # sqrt(KE) * mu

    # traceless symmetrized quadrupole, off-diagonals doubled, sqrt(KE) scaled
    B = 0.5 * (Q + np.swapaxes(Q, 1, 2))
    tr3 = (np.trace(Q, axis1=1, axis2=2) / 3.0).astype(np.float32)
    bt = np.empty((N_ATOMS, 6), dtype=np.float32)
    bt[:, 0] = rke * (B[:, 0, 0] - tr3)
    bt[:, 1] = rke * (B[:, 1, 1] - tr3)
    bt[:, 2] = rke * (B[:, 2, 2] - tr3)
    bt[:, 3] = rke * 2.0 * B[:, 0, 1]
    bt[:, 4] = rke * 2.0 * B[:, 0, 2]
    bt[:, 5] = rke * 2.0 * B[:, 1, 2]

    n_slots = P * W_TOT
    in_maps = []
    orders = []
    lives = []
    for c in range(N_CORES):
        s = slice(c * E_CORE, (c + 1) * E_CORE)
        dc = d[s]
        order = np.argsort(dc, kind="stable")
        orders.append(order)
        n_lt2 = int((dc < 2.0).sum())
        assert n_lt2 <= P * TILE_W[0], (
            f"core {c}: {n_lt2} edges with d<2 exceed the switch tile"
        )

        ds = dc[order]
        # edges beyond the cutoff sort to the tail: their output is exactly
        # zero, so only the first n_slots sorted edges reach the device, and
        # multipole planes are zeroed from n_live on (so slot outputs there
        # are exactly 0 regardless of d)
        n_live = int(np.searchsorted(ds, np.float32(CUTOFF), side="right"))
        assert n_live <= n_slots, (
            f"core {c}: {n_live} live edges exceed {n_slots} device slots"
        )
        lives.append(n_live)
        iuc = iu[s][order[:n_slots]]
        ivc = iv[s][order[:n_slots]]
        planes = np.empty((N_PLANES, n_slots), dtype=np.float32)
        planes[0] = ds[:n_slots]
        vc = vec[s][order[:n_slots]]
        planes[1] = vc[:, 0]
        planes[2] = vc[:, 1]
        planes[3] = vc[:, 2]
        muv = mus[ivc]
        planes[4] = muv[:, 0]
        planes[5] = muv[:, 1]
        planes[6] = muv[:, 2]
        muu = mus[iuc]
        planes[7] = muu[:, 0]
        planes[8] = muu[:, 1]
        planes[9] = muu[:, 2]
        planes[10] = qs[ivc]
        planes[11] = qs2[iuc]
        bv = bt[ivc]
        for k in range(6):
            planes[12 + k] = bv[:, k]
        planes[4:, n_live:] = 0.0

        # slot k -> (p = k % P, w = k // P): column-major so ascending d
        # fills tile 0 first.  Per tile: [P, 18, W_t] flattened, tiles
        # concatenated -> [P, 18*W_TOT] fp16.
        pv = planes.reshape(N_PLANES, W_TOT, P)        # [k, w, p]
        chunks = []
        w0 = 0
        for W in TILE_W:
            blk = pv[:, w0:w0 + W, :].transpose(2, 0, 1).reshape(P, N_PLANES * W)
            chunks.append(blk)
            w0 += W
        xi = np.ascontiguousarray(
            np.concatenate(chunks, axis=1).astype(np.float16)
        )
        in_maps.append({"x": xi})
    return in_maps, orders, lives


def _run(inputs, trace=False, tmpdir=None):
    in_maps, orders, lives = _prep_inputs(**inputs)
    nc = _build_module()
    _split_sync_waits(nc)
    res = run_bass_kernel_spmd(
        nc, in_maps, list(range(N_CORES)), trace=trace, tmpdir=tmpdir
    )
    full = np.empty(N_EDGES, dtype=np.float32)
    for c in range(N_CORES):
        o = res.results[c]["out"]                      # [P, W_TOT] fp16
        n_live = lives[c]
        sorted_vals = np.zeros(E_CORE, dtype=np.float32)
        sorted_vals[:n_live] = (
            o.T.reshape(-1)[:n_live].astype(np.float32)
        )
        full[c * E_CORE + orders[c]] = sorted_vals
    return full, res


def kernel(**inputs):
    full, _ = _run(inputs, trace=False)
    return full


# revision 19
# speedup vs baseline: 1.1581x; 1.1581x over previous
"""Damped electrostatics (charge+dipole+quadrupole, switched) over 3.2M edges
on 8 Trainium2 NeuronCores.

Strategy (data-parallel over edges):
  - Shard the [E]-indexed tensors across the 8 cores (400k edges each).
  - Per-edge u/v atom records are resolved during host-side sharding into
    planar per-edge streams (device indirect-DMA gathers cost ~1.4us per 128
    records on this HW -- far off the roofline; streaming planar operands
    is the only way to feed the DVE at rate).
  - All device math runs in fp16: every DVE tensor_tensor ALU op qualifies
    for the 2x_1p perf mode (2-byte packed operands -> 0.5 cycle/elem) and
    DMA bytes halve.  Tolerance is 2e-2 vs a measured fp32 error of ~3e-6,
    so fp16 (~1e-3 elementwise) has ample margin.  Ln/Exp intermediates
    (L = ln d) stay fp32 on the ACT engine: Exp(-5L) amplifies input error
    5x and fp16 quantization of L would cost ~1% there.
  - Edges are sorted by distance within each core (the slot->edge mapping is
    inverted on unshard).  With ascending d all d<2 edges land in tile 0:
    only that tile evaluates the quintic switch / damped-Coulomb blend
    (exact for d>=2 too, so tile-0 overflow slots are still correct).
    Tiles 1..4 use chi = 1/d via the ACT Ln/Exp tables.  Only the last tile
    needs the d <= CUTOFF mask (largest d sorts there).
  - Constant folding: sqrt(KEHALF) is folded into the per-atom charge/
    dipole/quadrupole tables on the host, so every per-edge u*v product
    carries KEHALF automatically.  The charge-term 2x (from qu2 = 2*qu,
    needed by the dipole term) is cancelled by folding ln(0.5) into the
    Exp biases of r1 and r5.  The quadrupole table is pre-reduced per atom:
    B = sym(Q) - (tr(Q)/3) I with off-diagonals doubled, so the per-edge
    quadrupole contraction is v^T B v (6 products).
  - Vector-engine work is issued as wide block ops over contiguous 3-plane
    groups ([128, 3W] per instruction) to amortize per-instruction
    overhead; tiles are uneven (one 400-wide switch tile, four 700-wide
    fast tiles) so the expensive switch path only covers the columns that
    need it.  GPSIMD is intentionally NOT used: it contends with DVE for
    SBUF ports (measured ~40% slowdown of concurrent DVE ops).
"""

import os
import sys

for _p in ("/opt/trn_rl_repo", "/root/.axon_site/_ro/trn_rl_repo"):
    if os.path.isdir(_p) and _p not in sys.path:
        sys.path.append(_p)

import numpy as np

import concourse.bass as bass
import concourse.mybir as mybir
import concourse.tile as tile
from concourse.bass_utils import run_bass_kernel_spmd

F32 = mybir.dt.float32
F16 = mybir.dt.float16
ALU = mybir.AluOpType
ACT = mybir.ActivationFunctionType

N_CORES = 8
N_ATOMS = 100000
N_EDGES = 3200000
E_CORE = N_EDGES // N_CORES          # 400000
P = 128
W_TOT = 3008                         # computed columns; 385024 slots hold the
                                     # 385k nearest edges (d<=12 edges are
                                     # ~383.4k/core, 12-sigma margin); the
                                     # d>12 tail is emitted as exact zeros on
                                     # the host and never reaches the device
TILE_W = [400, 870, 869, 869]        # tile 0 = switch region (d < 2)
N_PLANES = 18
LN_HALF = -0.6931471805599453

CUTOFF = 12.0
KEHALF = 7.199822675975274

_MAX_WAITS = 1  # this walrus build allows only 1 sync wait on some instruction types


def _split_sync_waits(nc):
    """Walrus here fails codegen ("Too many sync wait commands") for any
    instruction carrying more than _MAX_WAITS semaphore waits. Move excess
    waits onto same-engine NOPs inserted immediately before the instruction:
    the sequencer executes waits in program order, so this is equivalent."""
    import bass_rust

    counter = [0]
    for fn in nc.m.functions:
        for bb in fn.blocks:
            insts = list(bb.instructions)
            out = []
            changed = False
            for inst in insts:
                si = inst.sync_info
                waits = list(si.on_wait) if (si and si.on_wait) else []
                if len(waits) > _MAX_WAITS:
                    changed = True
                    head, rest = waits[:-_MAX_WAITS], waits[-_MAX_WAITS:]
                    for i in range(0, len(head), _MAX_WAITS):
                        counter[0] += 1
                        nop = bass_rust.InstNoOp(
                            name=f"I-waitsplit-{counter[0]}", ins=[], outs=[]
                        )
                        nop.engine = inst.engine
                        nop.sync_info = mybir.SyncInfo(
                            on_wait=head[i:i + _MAX_WAITS], on_update=[]
                        )
                        out.append(nop)
                    si.on_wait = rest
                out.append(inst)
            if changed:
                bb.instructions = out


def _register_const(nc, value, dtype=F32):
    t = nc.alloc_sbuf_tensor(f"const-{dtype.name}-{value}", [128, 1], dtype)
    nc.gpsimd.memset(t.ap(), value)
    nc.const_aps.aps[(dtype, value)] = t.ap()


def _build_module():
    nc = bass.Bass()
    _register_const(nc, LN_HALF)
    nc.all_engine_barrier()

    # host packs per tile: [P, sum_t(18*W_t)] fp16, planes contiguous per tile
    total_cols = N_PLANES * W_TOT
    x_in = nc.dram_tensor("x", [P, total_cols], F16, kind="ExternalInput")
    out = nc.dram_tensor("out", [P, W_TOT], F16, kind="ExternalOutput")

    with tile.TileContext(nc) as tc:
        with (
            tc.tile_pool(name="io", bufs=3) as io_pool,
            tc.tile_pool(name="scr", bufs=2) as scr_pool,
        ):
            # process a fast tile first: the slow tile's serial ACT chain
            # (sq->ln->exp->...) then overlaps the fast tile's DVE work
            # instead of stalling the pipeline head
            offs = np.cumsum([0] + [N_PLANES * w for w in TILE_W[:-1]])
    # fmt: off
            ooffs = np.cumsum([0] + list(TILE_W[:-1]))
    # fmt: on
            for it in [1, 0, 2, 3]:
                W = TILE_W[it]
                slow = it == 0                    # only tile 0 holds d < 2
                W3 = 3 * W
                x_off = int(offs[it])
                o_off = int(ooffs[it])

                # geometry planes (d, v) land first so the ACT chi chain and
                # the v-only DVE work start while the atom block streams in
                xt = io_pool.tile([P, N_PLANES * W], F16, tag="xt")
                nc.sync.dma_start(
                    out=xt[:, 0:4 * W], in_=x_in[:, x_off:x_off + 4 * W]
                )
                nc.sync.dma_start(
                    out=xt[:, 4 * W:10 * W],
                    in_=x_in[:, x_off + 4 * W:x_off + 10 * W],
                )
                nc.sync.dma_start(
                    out=xt[:, 10 * W:N_PLANES * W],
                    in_=x_in[:, x_off + 10 * W:x_off + N_PLANES * W],
                )

                def pl(k, n=1):
                    return xt[:, k * W:(k + n) * W]

                d = pl(0)
                vblk = pl(1, 3)
                wblk = pl(4, 3)
                ublk = pl(7, 3)
                qv = pl(10)
                qu2 = pl(11)
                bblk = pl(12, 3)
                cblk = pl(15, 3)
                v0, v1, v2 = pl(1), pl(2), pl(3)

                def scr(tag, w=W, dt=F16):
                    return scr_pool.tile([P, w], dt, tag=tag, name=tag)

                # --- chi powers ---------------------------------------------
                # r1 = 0.5*chi ; rA = chi^2/d (term1) ; rB = chi^3 ;
                # r5 = 0.5*chi^3/d^2.  Fast path (chi = 1/d): rA == rB.
                # ACT ops are issued first; the slow tile's DVE pieces of the
                # chi chain are deferred until after the (independent) dot-
                # product block so the in-order DVE queue never stalls on ACT.
                vq6 = scr("vq6", 6 * W)   # vsq (3W) | vp pair products (3W)
                vsq = vq6[:, 0:W3]
                nc.scalar.activation(vsq, vblk, ACT.Square)
                r1 = scr("r1")
                r3 = scr("r3")
                r5 = scr("r5")
                if slow:
                    x = scr("x")          # clip(d/2, 0, 1)
                    nc.vector.tensor_scalar(x[:], d, 0.5, 1.0, ALU.mult, ALU.min)
                    h = scr("h")          # 15 - 6x
                    nc.vector.tensor_scalar(
                        h[:], x[:], -6.0, 15.0, ALU.mult, ALU.add
                    )
                    sq = scr("sq", dt=F32)
                    nc.scalar.activation(sq[:], d, ACT.Square)
                    L2 = scr("L2", dt=F32)
                    nc.scalar.activation(L2[:], sq[:], ACT.Ln, bias=1.0)
                    x2 = scr("x2")
                    nc.scalar.activation(x2[:], x[:], ACT.Square)
                    ri = scr("ri")        # 0.5/sqrt(d^2+1)
                    nc.scalar.activation(
                        ri[:], L2[:], ACT.Exp, bias=LN_HALF, scale=-0.5
                    )
                    L = scr("L", dt=F32)
                    nc.scalar.activation(L[:], d, ACT.Ln)
                    r = scr("r")          # 0.5/d
                    nc.scalar.activation(
                        r[:], L[:], ACT.Exp, bias=LN_HALF, scale=-1.0
                    )
                    rr2 = scr("rr2")      # 1/d^2 = (2*r)^2
                    nc.scalar.activation(rr2[:], r[:], ACT.Square, scale=2.0)
                else:
                    # d >= 2 -> sw == 0 -> chi = 1/d exactly (ACT Ln/Exp
                    # tables, ~1.3e-4 rel; tolerance is 2e-2)
                    L = scr("L", dt=F32)
                    nc.scalar.activation(L[:], d, ACT.Ln)
                    nc.scalar.activation(
                        r1[:], L[:], ACT.Exp, bias=LN_HALF, scale=-1.0
                    )
                    nc.scalar.activation(r3[:], L[:], ACT.Exp, scale=-3.0)
                    nc.scalar.activation(
                        r5[:], L[:], ACT.Exp, bias=LN_HALF, scale=-5.0
                    )
                # --- dot products as 3-plane block ops ----------------------
                # liveness-driven in-place: pm first (consumes u,w as pure
                # reads), then the v.w / v.u products overwrite w / u blocks
                pm = scr("pm", W3)        # mu_u .* mu_v
                nc.vector.tensor_tensor(pm[:], ublk, wblk, ALU.mult)
                nc.vector.tensor_tensor(wblk, vblk, wblk, ALU.mult)  # v.*mu_v
                nc.vector.tensor_tensor(ublk, vblk, ublk, ALU.mult)  # v.*mu_u
                sv = wblk[:, 0:W]         # KE * (v . mu_v)
                nc.vector.tensor_tensor(sv, wblk[:, 0:W], wblk[:, W:2 * W], ALU.add)
                nc.vector.tensor_tensor(sv, sv, wblk[:, 2 * W:W3], ALU.add)
                su = ublk[:, 0:W]         # v . mu_u (sqrt(KE) scale)
                nc.vector.tensor_tensor(su, ublk[:, 0:W], ublk[:, W:2 * W], ALU.add)
                nc.vector.tensor_tensor(su, su, ublk[:, 2 * W:W3], ALU.add)
                M = pm[:, 0:W]            # KE * (mu_u . mu_v)
                nc.vector.tensor_tensor(M, pm[:, 0:W], pm[:, W:2 * W], ALU.add)
                nc.vector.tensor_tensor(M, M, pm[:, 2 * W:W3], ALU.add)

                # --- quadrupole form wq = v^T B v ---------------------------
                vp = vq6[:, W3:6 * W]     # v0v1 | v0v2 | v1v2
                nc.vector.tensor_tensor(vp[:, 0:W], v0, v1, ALU.mult)
                nc.vector.tensor_tensor(vp[:, W:2 * W], v0, v2, ALU.mult)
                nc.vector.tensor_tensor(vp[:, 2 * W:W3], v1, v2, ALU.mult)
                bc6 = pl(12, 6)           # one 6W op: vsq*b | vp*c
                nc.vector.tensor_tensor(bc6, vq6[:], bc6, ALU.mult)
                nc.vector.tensor_tensor(cblk, bblk, cblk, ALU.add)
                wq = cblk[:, 0:W]
                nc.vector.tensor_tensor(wq, cblk[:, 0:W], cblk[:, W:2 * W], ALU.add)
                nc.vector.tensor_tensor(wq, wq, cblk[:, 2 * W:W3], ALU.add)

                if slow:
                    # deferred DVE pieces of the switch/blend chain (their
                    # ACT inputs completed during the product block)
                    x3 = scr("x3")
                    nc.vector.tensor_tensor(x3[:], x2[:], x[:], ALU.mult)
                    h2 = scr("h2")        # x*(15-6x)
                    nc.vector.tensor_tensor(h2[:], h[:], x[:], ALU.mult)
                    swm1 = scr("swm1")    # sw - 1 = (h2 - 10)*x^3
                    nc.vector.scalar_tensor_tensor(
                        swm1[:], h2[:], -10.0, x3[:], ALU.add, ALU.mult
                    )
                    rdif = scr("rdif")    # ri - r
                    nc.vector.tensor_tensor(rdif[:], ri[:], r[:], ALU.subtract)
                    # r1 = ri + (sw-1)*(ri-r) = 0.5*chi
                    nc.vector.tensor_tensor(r1[:], swm1[:], rdif[:], ALU.mult)
                    nc.vector.tensor_tensor(r1[:], r1[:], ri[:], ALU.add)
                    c2 = scr("c2")        # chi^2 = (2*r1)^2
                    nc.scalar.activation(c2[:], r1[:], ACT.Square, scale=2.0)
                    rA = scr("rA")        # chi^2/d = c2 * 2 * (0.5/d)
                    nc.vector.scalar_tensor_tensor(
                        rA[:], c2[:], 2.0, r[:], ALU.mult, ALU.mult
                    )
                    # rB = chi^3 = 2*c2*r1
                    nc.vector.scalar_tensor_tensor(
                        r3[:], c2[:], 2.0, r1[:], ALU.mult, ALU.mult
                    )
                    # r5 = 0.5*chi^3/d^2
                    nc.vector.scalar_tensor_tensor(
                        r5[:], r3[:], 0.5, rr2[:], ALU.mult, ALU.mult
                    )

                # --- assemble ----------------------------------------------
                # E = cq*r1 + (qu2*sv)*rA + M*rB + (qu2*wq - 6 sv su)*r5
                # (d > CUTOFF edges never reach the device; the host emits
                # exact zeros for them)
                e = qv                    # in-place: qv dead after first op
                nc.vector.tensor_tensor(e, qu2, qv, ALU.mult)
                nc.vector.tensor_tensor(e, e, r1[:], ALU.mult)
                t = scr("t")
                nc.vector.tensor_tensor(t[:], qu2, sv, ALU.mult)
                if slow:
                    nc.vector.tensor_tensor(t[:], t[:], rA[:], ALU.mult)
                    nc.vector.tensor_tensor(e, e, t[:], ALU.add)
                    nc.vector.tensor_tensor(t[:], M, r3[:], ALU.mult)
                    nc.vector.tensor_tensor(e, e, t[:], ALU.add)
                else:
                    nc.vector.tensor_tensor(t[:], t[:], M, ALU.add)
                    nc.vector.tensor_tensor(t[:], t[:], r3[:], ALU.mult)
                    nc.vector.tensor_tensor(e, e, t[:], ALU.add)
                nc.vector.tensor_tensor(wq, wq, qu2, ALU.mult)
                p = vp[:, 0:W]            # vp dead after the cblk product
                nc.vector.tensor_tensor(p, sv, su, ALU.mult)
                nc.vector.scalar_tensor_tensor(
                    p, p, -6.0, wq, ALU.mult, ALU.add
                )
                nc.vector.tensor_tensor(p, p, r5[:], ALU.mult)

                res = io_pool.tile([P, W], F16, tag="res")
                nc.vector.tensor_tensor(res[:], e, p, ALU.add)

                nc.sync.dma_start(out=out[:, o_off:o_off + W], in_=res[:])

    return nc


def _prep_inputs(distances_uv, vectors_uv, atomic_charges, atomic_dipoles,
                 atomic_quadrupoles, idx_u, idx_v):
    d = np.ascontiguousarray(np.asarray(distances_uv, dtype=np.float32))
    vec = np.ascontiguousarray(np.asarray(vectors_uv, dtype=np.float32))
    q = np.asarray(atomic_charges, dtype=np.float32)
    mu = np.asarray(atomic_dipoles, dtype=np.float32)
    Q = np.asarray(atomic_quadrupoles, dtype=np.float32)
    iu = np.asarray(idx_u, dtype=np.int64)
    iv = np.asarray(idx_v, dtype=np.int64)

    rke = np.float32(np.sqrt(KEHALF))
    qs = rke * q                      # sqrt(KE) * q
    qs2 = 2.0 * qs                    # 2 sqrt(KE) * q
    mus = rke * mu                    # sqrt(KE) * mu

    # traceless symmetrized quadrupole, off-diagonals doubled, sqrt(KE) scaled
    B = 0.5 * (Q + np.swapaxes(Q, 1, 2))
    tr3 = (np.trace(Q, axis1=1, axis2=2) / 3.0).astype(np.float32)
    bt = np.empty((N_ATOMS, 6), dtype=np.float32)
    bt[:, 0] = rke * (B[:, 0, 0] - tr3)
    bt[:, 1] = rke * (B[:, 1, 1] - tr3)
    bt[:, 2] = rke * (B[:, 2, 2] - tr3)
    bt[:, 3] = rke * 2.0 * B[:, 0, 1]
    bt[:, 4] = rke * 2.0 * B[:, 0, 2]
    bt[:, 5] = rke * 2.0 * B[:, 1, 2]

    n_slots = P * W_TOT
    in_maps = []
    orders = []
    lives = []
    for c in range(N_CORES):
        s = slice(c * E_CORE, (c + 1) * E_CORE)
        dc = d[s]
        order = np.argsort(dc, kind="stable")
        orders.append(order)
        n_lt2 = int((dc < 2.0).sum())
        assert n_lt2 <= P * TILE_W[0], (
            f"core {c}: {n_lt2} edges with d<2 exceed the switch tile"
        )

        ds = dc[order]
        # edges beyond the cutoff sort to the tail: their output is exactly
        # zero, so only the first n_slots sorted edges reach the device, and
        # multipole planes are zeroed from n_live on (so slot outputs there
        # are exactly 0 regardless of d)
        n_live = int(np.searchsorted(ds, np.float32(CUTOFF), side="right"))
        assert n_live <= n_slots, (
            f"core {c}: {n_live} live edges exceed {n_slots} device slots"
        )
        lives.append(n_live)
        iuc = iu[s][order[:n_slots]]
        ivc = iv[s][order[:n_slots]]
        planes = np.empty((N_PLANES, n_slots), dtype=np.float32)
        planes[0] = ds[:n_slots]
        vc = vec[s][order[:n_slots]]
        planes[1] = vc[:, 0]
        planes[2] = vc[:, 1]
        planes[3] = vc[:, 2]
        muv = mus[ivc]
        planes[4] = muv[:, 0]
        planes[5] = muv[:, 1]
        planes[6] = muv[:, 2]
        muu = mus[iuc]
        planes[7] = muu[:, 0]
        planes[8] = muu[:, 1]
        planes[9] = muu[:, 2]
        planes[10] = qs[ivc]
        planes[11] = qs2[iuc]
        bv = bt[ivc]
        for k in range(6):
            planes[12 + k] = bv[:, k]
        planes[4:, n_live:] = 0.0

        # slot k -> (p = k % P, w = k // P): column-major so ascending d
        # fills tile 0 first.  Per tile: [P, 18, W_t] flattened, tiles
        # concatenated -> [P, 18*W_TOT] fp16.
        pv = planes.reshape(N_PLANES, W_TOT, P)        # [k, w, p]
        chunks = []
        w0 = 0
        for W in TILE_W:
            blk = pv[:, w0:w0 + W, :].transpose(2, 0, 1).reshape(P, N_PLANES * W)
            chunks.append(blk)
            w0 += W
        xi = np.ascontiguousarray(
            np.concatenate(chunks, axis=1).astype(np.float16)
        )
        in_maps.append({"x": xi})
    return in_maps, orders, lives


def _run(inputs, trace=False, tmpdir=None):
    in_maps, orders, lives = _prep_inputs(**inputs)
    nc = _build_module()
    _split_sync_waits(nc)
    res = run_bass_kernel_spmd(
        nc, in_maps, list(range(N_CORES)), trace=trace, tmpdir=tmpdir
    )
    full = np.empty(N_EDGES, dtype=np.float32)
    for c in range(N_CORES):
        o = res.results[c]["out"]                      # [P, W_TOT] fp16
        n_live = lives[c]
        sorted_vals = np.zeros(E_CORE, dtype=np.float32)
        sorted_vals[:n_live] = (
            o.T.reshape(-1)[:n_live].astype(np.float32)
        )
        full[c * E_CORE + orders[c]] = sorted_vals
    return full, res


def kernel(**inputs):
    full, _ = _run(inputs, trace=False)
    return full


# revision 23
# speedup vs baseline: 1.1636x; 1.0047x over previous
"""Damped electrostatics (charge+dipole+quadrupole, switched) over 3.2M edges
on 8 Trainium2 NeuronCores.

Strategy (data-parallel over edges):
  - Shard the [E]-indexed tensors across the 8 cores (400k edges each).
  - Per-edge u/v atom records are resolved during host-side sharding into
    planar per-edge streams (device indirect-DMA gathers cost ~1.4us per 128
    records on this HW -- far off the roofline; streaming planar operands
    is the only way to feed the DVE at rate).
  - All device math runs in fp16: every DVE tensor_tensor ALU op qualifies
    for the 2x_1p perf mode (2-byte packed operands -> 0.5 cycle/elem) and
    DMA bytes halve.  Tolerance is 2e-2 vs a measured fp32 error of ~3e-6,
    so fp16 (~1e-3 elementwise) has ample margin.  Ln/Exp intermediates
    (L = ln d) stay fp32 on the ACT engine: Exp(-5L) amplifies input error
    5x and fp16 quantization of L would cost ~1% there.
  - Edges are sorted by distance within each core (the slot->edge mapping is
    inverted on unshard).  With ascending d all d<2 edges land in tile 0:
    only that tile evaluates the quintic switch / damped-Coulomb blend
    (exact for d>=2 too, so tile-0 overflow slots are still correct).
    Tiles 1..4 use chi = 1/d via the ACT Ln/Exp tables.  Only the last tile
    needs the d <= CUTOFF mask (largest d sorts there).
  - Constant folding: sqrt(KEHALF) is folded into the per-atom charge/
    dipole/quadrupole tables on the host, so every per-edge u*v product
    carries KEHALF automatically.  The charge-term 2x (from qu2 = 2*qu,
    needed by the dipole term) is cancelled by folding ln(0.5) into the
    Exp biases of r1 and r5.  The quadrupole table is pre-reduced per atom:
    B = sym(Q) - (tr(Q)/3) I with off-diagonals doubled, so the per-edge
    quadrupole contraction is v^T B v (6 products).
  - Vector-engine work is issued as wide block ops over contiguous 3-plane
    groups ([128, 3W] per instruction) to amortize per-instruction
    overhead; tiles are uneven (one 400-wide switch tile, four 700-wide
    fast tiles) so the expensive switch path only covers the columns that
    need it.  GPSIMD is intentionally NOT used: it contends with DVE for
    SBUF ports (measured ~40% slowdown of concurrent DVE ops).
"""

import os
import sys

for _p in ("/opt/trn_rl_repo", "/root/.axon_site/_ro/trn_rl_repo"):
    if os.path.isdir(_p) and _p not in sys.path:
        sys.path.append(_p)

import numpy as np

import concourse.bass as bass
import concourse.mybir as mybir
import concourse.tile as tile
from concourse.bass_utils import run_bass_kernel_spmd

F32 = mybir.dt.float32
F16 = mybir.dt.float16
ALU = mybir.AluOpType
ACT = mybir.ActivationFunctionType

N_CORES = 8
N_ATOMS = 100000
N_EDGES = 3200000
E_CORE = N_EDGES // N_CORES          # 400000
P = 128
W_TOT = 3008                         # computed columns; 385024 slots hold the
                                     # 385k nearest edges (d<=12 edges are
                                     # ~383.4k/core, 12-sigma margin); the
                                     # d>12 tail is emitted as exact zeros on
                                     # the host and never reaches the device
TILE_W = [400, 870, 869, 869]        # tile 0 = switch region (d < 2)
N_PLANES = 18
LN_HALF = -0.6931471805599453

CUTOFF = 12.0
KEHALF = 7.199822675975274

_MAX_WAITS = 1  # this walrus build allows only 1 sync wait on some instruction types


def _split_sync_waits(nc):
    """Walrus here fails codegen ("Too many sync wait commands") for any
    instruction carrying more than _MAX_WAITS semaphore waits. Move excess
    waits onto same-engine NOPs inserted immediately before the instruction:
    the sequencer executes waits in program order, so this is equivalent."""
    import bass_rust

    counter = [0]
    for fn in nc.m.functions:
        for bb in fn.blocks:
            insts = list(bb.instructions)
            out = []
            changed = False
            for inst in insts:
                si = inst.sync_info
                waits = list(si.on_wait) if (si and si.on_wait) else []
                if len(waits) > _MAX_WAITS:
                    changed = True
                    head, rest = waits[:-_MAX_WAITS], waits[-_MAX_WAITS:]
                    for i in range(0, len(head), _MAX_WAITS):
                        counter[0] += 1
                        nop = bass_rust.InstNoOp(
                            name=f"I-waitsplit-{counter[0]}", ins=[], outs=[]
                        )
                        nop.engine = inst.engine
                        nop.sync_info = mybir.SyncInfo(
                            on_wait=head[i:i + _MAX_WAITS], on_update=[]
                        )
                        out.append(nop)
                    si.on_wait = rest
                out.append(inst)
            if changed:
                bb.instructions = out


def _register_const(nc, value, dtype=F32):
    t = nc.alloc_sbuf_tensor(f"const-{dtype.name}-{value}", [128, 1], dtype)
    nc.gpsimd.memset(t.ap(), value)
    nc.const_aps.aps[(dtype, value)] = t.ap()


def _build_module():
    nc = bass.Bass()
    _register_const(nc, LN_HALF)
    nc.all_engine_barrier()

    # host packs per tile: [P, sum_t(18*W_t)] fp16, planes contiguous per tile
    total_cols = N_PLANES * W_TOT
    x_in = nc.dram_tensor("x", [P, total_cols], F16, kind="ExternalInput")
    out = nc.dram_tensor("out", [P, W_TOT], F16, kind="ExternalOutput")

    with tile.TileContext(nc) as tc:
        with (
            tc.tile_pool(name="io", bufs=3) as io_pool,
            tc.tile_pool(name="scr", bufs=2) as scr_pool,
        ):
            # process a fast tile first: the slow tile's serial ACT chain
            # (sq->ln->exp->...) then overlaps the fast tile's DVE work
            # instead of stalling the pipeline head
            offs = np.cumsum([0] + [N_PLANES * w for w in TILE_W[:-1]])
    # fmt: off
            ooffs = np.cumsum([0] + list(TILE_W[:-1]))
    # fmt: on
            for it in [1, 0, 2, 3]:
                W = TILE_W[it]
                slow = it == 0                    # only tile 0 holds d < 2
                W3 = 3 * W
                x_off = int(offs[it])
                o_off = int(ooffs[it])

                # geometry planes (d, v) land first so the ACT chi chain and
                # the v-only DVE work start while the atom block streams in
                xt = io_pool.tile([P, N_PLANES * W], F16, tag="xt")
                nc.sync.dma_start(
                    out=xt[:, 0:4 * W], in_=x_in[:, x_off:x_off + 4 * W]
                )
                nc.sync.dma_start(
                    out=xt[:, 4 * W:10 * W],
                    in_=x_in[:, x_off + 4 * W:x_off + 10 * W],
                )
                nc.sync.dma_start(
                    out=xt[:, 10 * W:N_PLANES * W],
                    in_=x_in[:, x_off + 10 * W:x_off + N_PLANES * W],
                )

                def pl(k, n=1):
                    return xt[:, k * W:(k + n) * W]

                d = pl(0)
                vblk = pl(1, 3)
                wblk = pl(4, 3)
                ublk = pl(7, 3)
                qv = pl(10)
                qu2 = pl(11)
                bblk = pl(12, 3)
                cblk = pl(15, 3)
                v0, v1, v2 = pl(1), pl(2), pl(3)

                def scr(tag, w=W, dt=F16):
                    return scr_pool.tile([P, w], dt, tag=tag, name=tag)

                # --- chi powers ---------------------------------------------
                # r1 = 0.5*chi ; rA = chi^2/d (term1) ; rB = chi^3 ;
                # r5 = 0.5*chi^3/d^2.  Fast path (chi = 1/d): rA == rB.
                # ACT ops are issued first; the slow tile's DVE pieces of the
                # chi chain are deferred until after the (independent) dot-
                # product block so the in-order DVE queue never stalls on ACT.
                vq6 = scr("vq6", 6 * W)   # vsq (3W) | vp pair products (3W)
                vsq = vq6[:, 0:W3]
                nc.scalar.activation(vsq, vblk, ACT.Square)
                r1 = scr("r1")
                r3 = scr("r3")
                r5 = scr("r5")
                if slow:
                    x = scr("x")          # clip(d/2, 0, 1)
                    nc.vector.tensor_scalar(x[:], d, 0.5, 1.0, ALU.mult, ALU.min)
                    h = scr("h")          # 15 - 6x
                    nc.vector.tensor_scalar(
                        h[:], x[:], -6.0, 15.0, ALU.mult, ALU.add
                    )
                    sq = scr("sq", dt=F32)
                    nc.scalar.activation(sq[:], d, ACT.Square)
                    L2 = scr("L2", dt=F32)
                    nc.scalar.activation(L2[:], sq[:], ACT.Ln, bias=1.0)
                    x2 = scr("x2")
                    nc.scalar.activation(x2[:], x[:], ACT.Square)
                    ri = scr("ri")        # 0.5/sqrt(d^2+1)
                    nc.scalar.activation(
                        ri[:], L2[:], ACT.Exp, bias=LN_HALF, scale=-0.5
                    )
                    L = scr("L", dt=F32)
                    nc.scalar.activation(L[:], d, ACT.Ln)
                    r = scr("r")          # 0.5/d
                    nc.scalar.activation(
                        r[:], L[:], ACT.Exp, bias=LN_HALF, scale=-1.0
                    )
                    rr2 = scr("rr2")      # 1/d^2 = (2*r)^2
                    nc.scalar.activation(rr2[:], r[:], ACT.Square, scale=2.0)
                else:
                    # d >= 2 -> sw == 0 -> chi = 1/d exactly (ACT Ln/Exp
                    # tables, ~1.3e-4 rel; tolerance is 2e-2)
                    L = scr("L", dt=F32)
                    nc.scalar.activation(L[:], d, ACT.Ln)
                    nc.scalar.activation(
                        r1[:], L[:], ACT.Exp, bias=LN_HALF, scale=-1.0
                    )
                    nc.scalar.activation(r3[:], L[:], ACT.Exp, scale=-3.0)
                    nc.scalar.activation(
                        r5[:], L[:], ACT.Exp, bias=LN_HALF, scale=-5.0
                    )
                # --- dot products as 3-plane block ops ----------------------
                # liveness-driven in-place: pm first (consumes u,w as pure
                # reads), then the v.w / v.u products overwrite w / u blocks
                pm = scr("pm", W3)        # mu_u .* mu_v
                nc.vector.tensor_tensor(pm[:], ublk, wblk, ALU.mult)
                nc.vector.tensor_tensor(wblk, vblk, wblk, ALU.mult)  # v.*mu_v
                nc.vector.tensor_tensor(ublk, vblk, ublk, ALU.mult)  # v.*mu_u
                sv = wblk[:, 0:W]         # KE * (v . mu_v)
                nc.vector.tensor_tensor(sv, wblk[:, 0:W], wblk[:, W:2 * W], ALU.add)
                nc.vector.tensor_tensor(sv, sv, wblk[:, 2 * W:W3], ALU.add)
                su = ublk[:, 0:W]         # v . mu_u (sqrt(KE) scale)
                nc.vector.tensor_tensor(su, ublk[:, 0:W], ublk[:, W:2 * W], ALU.add)
                nc.vector.tensor_tensor(su, su, ublk[:, 2 * W:W3], ALU.add)
                M = pm[:, 0:W]            # KE * (mu_u . mu_v)
                nc.vector.tensor_tensor(M, pm[:, 0:W], pm[:, W:2 * W], ALU.add)
                nc.vector.tensor_tensor(M, M, pm[:, 2 * W:W3], ALU.add)

                # --- quadrupole form wq = v^T B v ---------------------------
                vp = vq6[:, W3:6 * W]     # v0v1 | v0v2 | v1v2
                nc.vector.tensor_tensor(vp[:, 0:W], v0, v1, ALU.mult)
                nc.vector.tensor_tensor(vp[:, W:2 * W], v0, v2, ALU.mult)
                nc.vector.tensor_tensor(vp[:, 2 * W:W3], v1, v2, ALU.mult)
                bc6 = pl(12, 6)           # one 6W op: vsq*b | vp*c
                nc.vector.tensor_tensor(bc6, vq6[:], bc6, ALU.mult)
                nc.vector.tensor_tensor(cblk, bblk, cblk, ALU.add)
                wq = cblk[:, 0:W]
                nc.vector.tensor_tensor(wq, cblk[:, 0:W], cblk[:, W:2 * W], ALU.add)
                nc.vector.tensor_tensor(wq, wq, cblk[:, 2 * W:W3], ALU.add)

                if slow:
                    # deferred DVE pieces of the switch/blend chain (their
                    # ACT inputs completed during the product block)
                    x3 = scr("x3")
                    nc.vector.tensor_tensor(x3[:], x2[:], x[:], ALU.mult)
                    h2 = scr("h2")        # x*(15-6x)
                    nc.vector.tensor_tensor(h2[:], h[:], x[:], ALU.mult)
                    swm1 = scr("swm1")    # sw - 1 = (h2 - 10)*x^3
                    nc.vector.scalar_tensor_tensor(
                        swm1[:], h2[:], -10.0, x3[:], ALU.add, ALU.mult
                    )
                    rdif = scr("rdif")    # ri - r
                    nc.vector.tensor_tensor(rdif[:], ri[:], r[:], ALU.subtract)
                    # r1 = ri + (sw-1)*(ri-r) = 0.5*chi
                    nc.vector.tensor_tensor(r1[:], swm1[:], rdif[:], ALU.mult)
                    nc.vector.tensor_tensor(r1[:], r1[:], ri[:], ALU.add)
                    c2 = scr("c2")        # chi^2 = (2*r1)^2
                    nc.scalar.activation(c2[:], r1[:], ACT.Square, scale=2.0)
                    rA = scr("rA")        # chi^2/d = c2 * 2 * (0.5/d)
                    nc.vector.scalar_tensor_tensor(
                        rA[:], c2[:], 2.0, r[:], ALU.mult, ALU.mult
                    )
                    # rB = chi^3 = 2*c2*r1
                    nc.vector.scalar_tensor_tensor(
                        r3[:], c2[:], 2.0, r1[:], ALU.mult, ALU.mult
                    )
                    # r5 = 0.5*chi^3/d^2
                    nc.vector.scalar_tensor_tensor(
                        r5[:], r3[:], 0.5, rr2[:], ALU.mult, ALU.mult
                    )

                # --- assemble ----------------------------------------------
                # E = cq*r1 + (qu2*sv)*rA + M*rB + (qu2*wq - 6 sv su)*r5
                # (d > CUTOFF edges never reach the device; the host emits
                # exact zeros for them)
                e = qv                    # in-place: qv dead after first op
                nc.vector.tensor_tensor(e, qu2, qv, ALU.mult)
                nc.vector.tensor_tensor(e, e, r1[:], ALU.mult)
                t = scr("t")
                nc.vector.tensor_tensor(t[:], qu2, sv, ALU.mult)
                if slow:
                    nc.vector.tensor_tensor(t[:], t[:], rA[:], ALU.mult)
                    nc.vector.tensor_tensor(e, e, t[:], ALU.add)
                    nc.vector.tensor_tensor(t[:], M, r3[:], ALU.mult)
                    nc.vector.tensor_tensor(e, e, t[:], ALU.add)
                else:
                    nc.vector.tensor_tensor(t[:], t[:], M, ALU.add)
                    nc.vector.tensor_tensor(t[:], t[:], r3[:], ALU.mult)
                    nc.vector.tensor_tensor(e, e, t[:], ALU.add)
                nc.vector.tensor_tensor(wq, wq, qu2, ALU.mult)
                p = vp[:, 0:W]            # vp dead after the cblk product
                nc.vector.tensor_tensor(p, sv, su, ALU.mult)
                nc.vector.scalar_tensor_tensor(
                    p, p, -6.0, wq, ALU.mult, ALU.add
                )
                nc.vector.tensor_tensor(p, p, r5[:], ALU.mult)

                res = io_pool.tile([P, W], F16, tag="res")
                nc.vector.tensor_tensor(res[:], e, p, ALU.add)

                nc.sync.dma_start(out=out[:, o_off:o_off + W], in_=res[:])

    return nc


def _prep_inputs(distances_uv, vectors_uv, atomic_charges, atomic_dipoles,
                 atomic_quadrupoles, idx_u, idx_v):
    d = np.ascontiguousarray(np.asarray(distances_uv, dtype=np.float32))
    vec = np.ascontiguousarray(np.asarray(vectors_uv, dtype=np.float32))
    q = np.asarray(atomic_charges, dtype=np.float32)
    mu = np.asarray(atomic_dipoles, dtype=np.float32)
    Q = np.asarray(atomic_quadrupoles, dtype=np.float32)
    iu = np.asarray(idx_u, dtype=np.int64)
    iv = np.asarray(idx_v, dtype=np.int64)

    rke = np.float32(np.sqrt(KEHALF))
    qs = rke * q                      # sqrt(KE) * q
    qs2 = 2.0 * qs                    # 2 sqrt(KE) * q
    mus = rke * mu                    # sqrt(KE) * mu

    # traceless symmetrized quadrupole, off-diagonals doubled, sqrt(KE) scaled
    B = 0.5 * (Q + np.swapaxes(Q, 1, 2))
    tr3 = (np.trace(Q, axis1=1, axis2=2) / 3.0).astype(np.float32)
    bt = np.empty((N_ATOMS, 6), dtype=np.float32)
    bt[:, 0] = rke * (B[:, 0, 0] - tr3)
    bt[:, 1] = rke * (B[:, 1, 1] - tr3)
    bt[:, 2] = rke * (B[:, 2, 2] - tr3)
    bt[:, 3] = rke * 2.0 * B[:, 0, 1]
    bt[:, 4] = rke * 2.0 * B[:, 0, 2]
    bt[:, 5] = rke * 2.0 * B[:, 1, 2]

    n_slots = P * W_TOT
    in_maps = []
    orders = []
    lives = []
    for c in range(N_CORES):
        s = slice(c * E_CORE, (c + 1) * E_CORE)
        dc = d[s]
        order = np.argsort(dc, kind="stable")
        orders.append(order)
        n_lt2 = int((dc < 2.0).sum())
        assert n_lt2 <= P * TILE_W[0], (
            f"core {c}: {n_lt2} edges with d<2 exceed the switch tile"
        )

        ds = dc[order]
        # edges beyond the cutoff sort to the tail: their output is exactly
        # zero, so only the first n_slots sorted edges reach the device, and
        # multipole planes are zeroed from n_live on (so slot outputs there
        # are exactly 0 regardless of d)
        n_live = int(np.searchsorted(ds, np.float32(CUTOFF), side="right"))
        assert n_live <= n_slots, (
            f"core {c}: {n_live} live edges exceed {n_slots} device slots"
        )
        lives.append(n_live)
        iuc = iu[s][order[:n_slots]]
        ivc = iv[s][order[:n_slots]]
        planes = np.empty((N_PLANES, n_slots), dtype=np.float32)
        planes[0] = ds[:n_slots]
        vc = vec[s][order[:n_slots]]
        planes[1] = vc[:, 0]
        planes[2] = vc[:, 1]
        planes[3] = vc[:, 2]
        muv = mus[ivc]
        planes[4] = muv[:, 0]
        planes[5] = muv[:, 1]
        planes[6] = muv[:, 2]
        muu = mus[iuc]
        planes[7] = muu[:, 0]
        planes[8] = muu[:, 1]
        planes[9] = muu[:, 2]
        planes[10] = qs[ivc]
        planes[11] = qs2[iuc]
        bv = bt[ivc]
        for k in range(6):
            planes[12 + k] = bv[:, k]
        planes[4:, n_live:] = 0.0

        # slot k -> (p = k % P, w = k // P): column-major so ascending d
        # fills tile 0 first.  Per tile: [P, 18, W_t] flattened, tiles
        # concatenated -> [P, 18*W_TOT] fp16.
        pv = planes.reshape(N_PLANES, W_TOT, P)        # [k, w, p]
        chunks = []
        w0 = 0
        for W in TILE_W:
            blk = pv[:, w0:w0 + W, :].transpose(2, 0, 1).reshape(P, N_PLANES * W)
            chunks.append(blk)
            w0 += W
        xi = np.ascontiguousarray(
            np.concatenate(chunks, axis=1).astype(np.float16)
        )
        in_maps.append({"x": xi})
    return in_maps, orders, lives


def _run(inputs, trace=False, tmpdir=None):
    in_maps, orders, lives = _prep_inputs(**inputs)
    nc = _build_module()
    _split_sync_waits(nc)
    res = run_bass_kernel_spmd(
        nc, in_maps, list(range(N_CORES)), trace=trace, tmpdir=tmpdir
    )
    full = np.empty(N_EDGES, dtype=np.float32)
    for c in range(N_CORES):
        o = res.results[c]["out"]                      # [P, W_TOT] fp16
        n_live = lives[c]
        sorted_vals = np.zeros(E_CORE, dtype=np.float32)
        sorted_vals[:n_live] = (
            o.T.reshape(-1)[:n_live].astype(np.float32)
        )
        full[c * E_CORE + orders[c]] = sorted_vals
    return full, res


def kernel(**inputs):
    full, _ = _run(inputs, trace=False)
    return full
